# revision 4
# baseline (speedup 1.0000x reference)
"""CrossAttention (B=2, N=4096, D=320, H=8, DH=64) on 8 trn2 NeuronCores.

Sharding: head-parallel. Core c computes head h=c for both batch elements.
Each core receives the full (pre-transposed) activations xT = x^T and its
head's weight slices; there is no cross-device communication.

Device-side math per core (per batch element b):
  qT = Wq_h^T @ xT[b]          [64, N]   (f32r / FP22 matmuls, full rate)
  kT = Wk_h^T @ xT[b]          [64, N]
  vT = Wv_h^T @ xT[b]          [64, N]   -> PE-transposed to natural v [N, 64]
  v' = [v | 1]                 [N, 65]   (ones column appended)
  S^T tiles = k @ qT           [128 keys, nq]   (PSUM)
  E^T = exp(S^T / 8)           (ACT engine, scale folded into activation)
  O'^T = v'^T @ E^T            [65, nq]  accumulated over key tiles in PSUM;
                               row 64 is the softmax denominator sum.
  p = O_unnorm @ Wo_h          [N, 320]  (unnormalized output projection)

Host side: gathers per-core outputs, transposes qT/kT/vT to (bh, n, d),
normalizes O by the denominators, and sums the per-head projection partials
plus the output bias (in float64 for a cleaner reduction).
"""

import numpy as np

B, N, D, H, DH = 2, 4096, 320, 8, 64
NCORES = 8
P = 128
NQ_SUPER = 1024           # exp chunk width = 2 PSUM banks
KCH = [(0, 128), (128, 128), (256, 64)]   # D=320 contraction chunks
SCALE = DH ** -0.5

_cache = {}


def _build_nc(n=N):
    import concourse.bacc as bacc
    import concourse.tile as tile
    import concourse.mybir as mybir
    from contextlib import ExitStack
    from concourse.masks import make_identity

    f32 = mybir.dt.float32
    f32r = mybir.dt.float32r
    EXP = mybir.ActivationFunctionType.Exp
    nkt = n // P              # number of 128-row key tiles

    nc = bacc.Bacc("TRN2", target_bir_lowering=False, debug=False,
                   enable_asserts=False)

    xT = nc.dram_tensor("xT", [B, D, n], f32r, kind="ExternalInput").ap()
    wq = nc.dram_tensor("wq", [D, DH], f32r, kind="ExternalInput").ap()
    wk = nc.dram_tensor("wk", [D, DH], f32r, kind="ExternalInput").ap()
    wv = nc.dram_tensor("wv", [D, DH], f32r, kind="ExternalInput").ap()
    wo = nc.dram_tensor("wo", [DH, D], f32r, kind="ExternalInput").ap()

    qT_out = nc.dram_tensor("qT_out", [B, DH, n], f32r, kind="ExternalOutput").ap()
    kT_out = nc.dram_tensor("kT_out", [B, DH, n], f32r, kind="ExternalOutput").ap()
    vT_out = nc.dram_tensor("vT_out", [B, DH, n], f32, kind="ExternalOutput").ap()
    oT_out = nc.dram_tensor("oT_out", [B, DH + 1, n], f32r, kind="ExternalOutput").ap()
    p_out = nc.dram_tensor("p_out", [B, n, D], f32, kind="ExternalOutput").ap()

    with tile.TileContext(nc) as tc, ExitStack() as ctx:
        const = ctx.enter_context(tc.tile_pool(name="const", bufs=1))
        xpool = ctx.enter_context(tc.tile_pool(name="xpool", bufs=1))
        qkv = ctx.enter_context(tc.tile_pool(name="qkv", bufs=1))
        epool = ctx.enter_context(tc.tile_pool(name="epool", bufs=3))
        opool = ctx.enter_context(tc.tile_pool(name="opool", bufs=4))
        popool = ctx.enter_context(tc.tile_pool(name="popool", bufs=3))
        s_ps = ctx.enter_context(tc.tile_pool(name="s_ps", bufs=2, space="PSUM"))
        pv_ps = ctx.enter_context(tc.tile_pool(name="pv_ps", bufs=1, space="PSUM"))
        m_ps = ctx.enter_context(tc.tile_pool(name="m_ps", bufs=2, space="PSUM"))

        ident = const.tile([P, P], f32)
        make_identity(nc, ident)

        def load_w(w, name):
            wsb = const.tile([P, 3 * DH], f32r, name=name)
            for ki, (k0, kw) in enumerate(KCH):
                nc.sync.dma_start(out=wsb[0:kw, ki * DH:(ki + 1) * DH],
                                  in_=w[k0:k0 + kw, :])
            return wsb

        wq_sb = load_w(wq, "wq_sb")
        wk_sb = load_w(wk, "wk_sb")
        wv_sb = load_w(wv, "wv_sb")
        wo_sb = const.tile([DH, D], f32r)
        nc.sync.dma_start(out=wo_sb, in_=wo)
        ones_c = const.tile([P, 1], f32)
        nc.vector.memset(ones_c, 1.0)

        for b in range(B):
            # ---------------- projections ----------------
            xts = []
            for ki, (k0, kw) in enumerate(KCH):
                xt = xpool.tile([P, n], f32r, name=f"xt{ki}", tag=f"xt{ki}")
                nc.sync.dma_start(out=xt[0:kw, :], in_=xT[b, k0:k0 + kw, :])
                xts.append(xt)

            def project(wsb, name, dt=f32r):
                tsb = qkv.tile([DH, n], dt, name=name, tag=name)
                for j in range(n // 512):
                    ps = m_ps.tile([DH, 512], f32, name="proj_ps", tag="mps")
                    for ki, (k0, kw) in enumerate(KCH):
                        nc.tensor.matmul(
                            ps,
                            lhsT=wsb[0:kw, ki * DH:(ki + 1) * DH],
                            rhs=xts[ki][0:kw, j * 512:(j + 1) * 512],
                            start=(ki == 0), stop=(ki == len(KCH) - 1))
                    nc.vector.tensor_copy(out=tsb[:, j * 512:(j + 1) * 512], in_=ps)
                return tsb

            qT_sb = project(wq_sb, "qT_sb")
            kT_sb = project(wk_sb, "kT_sb")
            vT_sb = project(wv_sb, "vT_sb", dt=f32)
            nc.sync.dma_start(out=qT_out[b], in_=qT_sb)
            nc.sync.dma_start(out=kT_out[b], in_=kT_sb)
            nc.sync.dma_start(out=vT_out[b], in_=vT_sb)

            # ------------- v natural (+ ones column) -------------
            v_sb = qkv.tile([P, nkt, DH + 1], f32r, tag="v_sb")
            nc.vector.tensor_copy(out=v_sb[:, :, DH],
                                  in_=ones_c.broadcast_to([P, nkt]))
            for i in range(nkt):
                tps = m_ps.tile([P, DH], f32, name="t_ps", tag="mps")
                nc.tensor.transpose(tps, vT_sb[:, i * P:(i + 1) * P],
                                    ident[0:DH, 0:DH])
                nc.vector.tensor_copy(out=v_sb[:, i, 0:DH], in_=tps)

            # ---------------- attention ----------------
            for sup0 in range(0, n, NQ_SUPER):
                width = min(NQ_SUPER, n - sup0)
                halves = width // 512
                pvs = [pv_ps.tile([DH + 1, 512], f32, name=f"pv{hh}", tag=f"pv{hh}")
                       for hh in range(halves)]
                prev = None
                for i in range(nkt):
                    sps = s_ps.tile([P, width], f32, name="s_tile", tag="sps")
                    for hh in range(halves):
                        nc.tensor.matmul(
                            sps[:, hh * 512:(hh + 1) * 512],
                            lhsT=kT_sb[:, i * P:(i + 1) * P],
                            rhs=qT_sb[:, sup0 + hh * 512: sup0 + (hh + 1) * 512]
                                ,
                            start=True, stop=True)
                    et = epool.tile([P, width], f32r, name="e_tile", tag="et")
                    nc.scalar.activation(out=et, in_=sps, func=EXP, scale=SCALE)

                    if prev is not None:
                        _emit_pv(nc, f32r, pvs, v_sb, prev, nkt, halves)
                    prev = (i, et)
                _emit_pv(nc, f32r, pvs, v_sb, prev, nkt, halves)

                # ------------- finalize this query block -------------
                for hh in range(halves):
                    osb = opool.tile([DH + 1, 512], f32r, name="o_tile", tag="osb")
                    nc.vector.tensor_copy(out=osb, in_=pvs[hh])
                    q0 = sup0 + hh * 512
                    nc.sync.dma_start(out=oT_out[b][:, q0:q0 + 512], in_=osb)
                    for t in range(4):
                        pps = m_ps.tile([P, D], f32, name="po_ps", tag="mps")
                        nc.tensor.matmul(
                            pps,
                            lhsT=osb[0:DH, t * P:(t + 1) * P],
                            rhs=wo_sb,
                            start=True, stop=True)
                        posb = popool.tile([P, D], f32, name="po_sb", tag="posb")
                        nc.vector.tensor_copy(out=posb, in_=pps)
                        nc.sync.dma_start(
                            out=p_out[b][q0 + t * P: q0 + (t + 1) * P, :],
                            in_=posb)

    nc.compile()
    return nc


def _emit_pv(nc, f32r, pvs, v_sb, prev, nkt, halves):
    pi, pe = prev
    for hh in range(halves):
        nc.tensor.matmul(
            pvs[hh],
            lhsT=v_sb[:, pi, :],
            rhs=pe[:, hh * 512:(hh + 1) * 512],
            start=(pi == 0), stop=(pi == nkt - 1))


def kernel(**inputs):
    from concourse.bass_utils import run_bass_kernel_spmd

    x = np.ascontiguousarray(np.asarray(inputs["x"], dtype=np.float32))
    Wq = np.asarray(inputs["Wq"], dtype=np.float32)
    Wk = np.asarray(inputs["Wk"], dtype=np.float32)
    Wv = np.asarray(inputs["Wv"], dtype=np.float32)
    Wo = np.asarray(inputs["Wo"], dtype=np.float32)
    bo = np.asarray(inputs["bo"], dtype=np.float32)

    if "nc" not in _cache:
        _cache["nc"] = _build_nc()
    nc = _cache["nc"]

    xT = np.ascontiguousarray(x.transpose(0, 2, 1))
    in_maps = []
    for c in range(NCORES):
        sl = slice(c * DH, (c + 1) * DH)
        in_maps.append({
            "xT": xT,
            "wq": np.ascontiguousarray(Wq[:, sl]),
            "wk": np.ascontiguousarray(Wk[:, sl]),
            "wv": np.ascontiguousarray(Wv[:, sl]),
            "wo": np.ascontiguousarray(Wo[sl, :]),
        })

    res = run_bass_kernel_spmd(nc, in_maps, core_ids=list(range(NCORES)))
    _cache["last_results"] = res
    rs = res.results

    qf = np.empty((B * H, N, DH), np.float32)
    kf = np.empty_like(qf)
    vf = np.empty_like(qf)
    inter = np.empty((B, H, N, DH), np.float32)
    final = np.zeros((B, N, D), np.float64)
    for c in range(NCORES):
        r = rs[c]
        for b in range(B):
            qf[b * H + c] = r["qT_out"][b].T
            kf[b * H + c] = r["kT_out"][b].T
            vf[b * H + c] = r["vT_out"][b].T
            sums = r["oT_out"][b][DH].astype(np.float64)            # (N,)
            inter[b, c] = (r["oT_out"][b][0:DH] / sums[None, :]).T
            final[b] += r["p_out"][b].astype(np.float64) / sums[:, None]
    final = (final + bo.astype(np.float64)).astype(np.float32)
    return final, qf, kf, vf, inter
